# revision 30
# baseline (speedup 1.0000x reference)
"""PointNet Feature Propagation kernel for Trainium2 (8 NeuronCores, SPMD).

Data-parallel over N: each core owns 2048 of 16384 points; xyz2/points2 and
the conv weights are replicated. Per core, per 128-point tile:

  1. Distance matrix via one K=21 bf16 matmul group per 512-col chunk: the
     hi/mid/lo bf16 coordinate splits and all significant cross terms are
     PRECOMPUTED ON THE HOST and DMA'd in as two [24, *] "aug" operand tiles,
     so psum = 2<x1,x2> - |x2|^2 to ~1e-7 with zero on-device prep.
  2. The psum chunks are drained to SBUF fp32 by the Scalar engine (frees
     PSUM banks for the interp/MLP matmuls that overlap the scan phase).
  3. DVE max8 + max_index over the [128, 2048] row give exact top-3 refs.
     This pair of full scans is the pacing cost of the kernel.
  4. Neighbor features are fetched with three per-tile indirect DMAs fired
     right after each tile's scan. HW SWDGE descriptor generation costs
     ~9ns/row regardless of batching (~53us Pool total for 48x384 rows),
     which still fits under the DVE scan pace when fully pipelined.
  5. Inverse-distance weights -> diagonal weight tiles built with zero
     engine cost: weights are DMA-scattered at stride 129 into a zeroed,
     padded DRAM buffer and DMA'd back as 128x128 diagonal matrices. PE
     identity-matmuls transpose + weight + k-sum the gathered features
     into feature-major interp chunks.
  6. Two 1x1-conv layers as bf16 PE matmuls with BN folded into the weights
     and biases on the host; bias+ReLU fused into one ACT op per chunk.
     Per-batch MLP work is emitted with a 2-tile lag so it hides inside the
     DVE scan phase.
"""
import ml_dtypes
import numpy as np

import concourse.bacc as bacc
import concourse.bass as bass
import concourse.mybir as mybir
from concourse import bass_utils
from concourse.tile import TileContext

f32 = mybir.dt.float32
bf16 = mybir.dt.bfloat16
u32 = mybir.dt.uint32

NCORES = 8
N = 16384
NLOC = N // NCORES          # 2048 points per core
S = 2048                    # reference points (replicated)
D1 = 128                    # points1 channels
D2 = 256                    # points2 channels
M0 = 256                    # mlp hidden
M1 = 128                    # mlp out
NT = NLOC // 128            # 16 point-tiles per core
NB = 4                      # tiles per gather/interp/mlp batch
NBATCH = NT // NB           # 4 batches
BPTS = NB * 128             # 512 points per batch
BN_EPS = 1e-5

_CACHE = {}


def build():
    nc = bacc.Bacc("TRN2", target_bir_lowering=False)

    aug1h = nc.dram_tensor("aug1h", [24, NLOC], bf16, kind="ExternalInput")
    aug2h = nc.dram_tensor("aug2h", [24, S], bf16, kind="ExternalInput")
    sq1h = nc.dram_tensor("sq1h", [128, NT, 3], f32, kind="ExternalInput")
    p1h = nc.dram_tensor("p1h", [D1, NLOC], bf16, kind="ExternalInput")
    p2bf = nc.dram_tensor("p2bf", [S, D2], bf16, kind="ExternalInput")
    w0h = nc.dram_tensor("w0h", [128, 3, M0], bf16, kind="ExternalInput")
    w1h = nc.dram_tensor("w1h", [128, 2, M1], bf16, kind="ExternalInput")
    b0h = nc.dram_tensor("b0h", [128, 2], f32, kind="ExternalInput")
    b1h = nc.dram_tensor("b1h", [128, 1], f32, kind="ExternalInput")
    dgz = nc.dram_tensor("dgz", [NT * 3, 128 * 129], bf16, kind="ExternalInput")
    out = nc.dram_tensor("out", [M1, NLOC], f32, kind="ExternalOutput")

    AL = mybir.AluOpType
    AX = mybir.AxisListType
    ACT = mybir.ActivationFunctionType

    with TileContext(nc) as tc:
        with tc.tile_pool(name="const", bufs=1) as cp:
            aug1 = cp.tile([24, NLOC], bf16)
            aug2 = cp.tile([24, S], bf16)
            sq1e = cp.tile([128, NT, 3], f32)
            p1b = cp.tile([D1, NLOC], bf16)
            w0b = cp.tile([128, 3, M0], bf16)
            w1b = cp.tile([128, 2, M1], bf16)
            b0s = cp.tile([128, 2], f32)
            b1s = cp.tile([128, 1], f32)

            # aug operands first: they gate the distance matmuls
            nc.sync.dma_start(aug1[:], aug1h[:])
            nc.sync.dma_start(aug2[:], aug2h[:])
            nc.sync.dma_start(sq1e[:], sq1h[:])
            nc.scalar.dma_start(p1b[:], p1h[:])
            nc.scalar.dma_start(w0b[:], w0h[:])
            nc.scalar.dma_start(w1b[:], w1h[:])
            nc.scalar.dma_start(b0s[:], b0h[:])
            nc.scalar.dma_start(b1s[:], b1h[:])

            mx = cp.tile([128, NT, 8], f32)
            g_all = cp.tile([128, NT * 3, D2], bf16)
            d3 = cp.tile([128, NT, 3], f32)
            w3f = cp.tile([128, NT, 3], f32)
            ws = cp.tile([128, NT], f32)
            interp = cp.tile([128, 2, NLOC], bf16)
            h0 = cp.tile([128, 2, NLOC], bf16)
            outsb = cp.tile([128, NLOC], f32)

            with tc.tile_pool(name="scan", bufs=4) as sp, \
                 tc.tile_pool(name="wrp", bufs=2) as wrp, \
                 tc.tile_pool(name="ixp", bufs=6) as ixpool, \
                 tc.tile_pool(name="dps", bufs=2, space="PSUM") as dps, \
                 tc.tile_pool(name="ips", bufs=2, space="PSUM") as ips, \
                 tc.tile_pool(name="mps", bufs=2, space="PSUM") as mps:

                dgs = [None] * NBATCH
                ixts = [None] * NT

                def knn_tile(t):
                    """dist matmuls -> ACT drain -> DVE top-8 scan -> gathers."""
                    sc = sp.tile([128, S], f32, tag="sc")
                    for half in range(2):
                        pc = dps.tile([128, 1024], f32, tag="pc")
                        for b2 in range(2):
                            col = 1024 * half + 512 * b2
                            nc.tensor.matmul(
                                pc[:, 512 * b2:512 * (b2 + 1)],
                                lhsT=aug1[0:21, 128 * t:128 * (t + 1)],
                                rhs=aug2[0:21, col:col + 512],
                                start=True, stop=True)
                        nc.scalar.copy(sc[:, 1024 * half:1024 * (half + 1)], pc[:])
                    nc.vector.max(out=mx[:, t, :], in_=sc[:])
                    # per-tile index tile: keeps the next tile's max_index
                    # write from WAR-waiting on this tile's gather reads
                    ixt = ixpool.tile([128, 8], u32, tag="ix")
                    ixts[t] = ixt
                    nc.vector.max_index(out=ixt[:], in_max=mx[:, t, :],
                                        in_values=sc[:])
                    # fire the feature gathers right away (Pool desc-gen is
                    # ~9ns/row on HW; pipelines under the next tiles' scans)
                    for k in range(3):
                        nc.gpsimd.indirect_dma_start(
                            out=g_all[:, 3 * t + k, :], out_offset=None,
                            in_=p2bf[:],
                            in_offset=bass.IndirectOffsetOnAxis(
                                ap=ixt[:, k:k + 1], axis=0))

                def batch_post_a(b):
                    """Weights + diag scatter (right after batch's scans)."""
                    t0, t1 = NB * b, NB * (b + 1)
                    sl = slice(t0, t1)
                    # inverse-distance weights (small batched DVE ops);
                    # sq1e comes host-side with +1e-8 folded in
                    nc.vector.tensor_tensor(
                        d3[:, sl, :], sq1e[:, sl, :], mx[:, sl, 0:3],
                        op=AL.subtract)
                    nc.vector.reciprocal(w3f[:, sl, :], d3[:, sl, :])
                    nc.vector.reduce_sum(ws[:, sl], w3f[:, sl, :], axis=AX.X)
                    nc.vector.reciprocal(ws[:, sl], ws[:, sl])
                    nc.vector.tensor_tensor(
                        w3f[:, sl, :], w3f[:, sl, :],
                        ws[:, sl].to_broadcast([128, NB, 3]), op=AL.mult)
                    # per-batch bf16 weight tile: the next batch's cast must
                    # not WAR-wait on this batch's scatter DMA read
                    w3bt = wrp.tile([128, NB, 3], bf16, tag="w3b")
                    nc.vector.tensor_copy(w3bt[:], w3f[:, sl, :])
                    # diag scatter: w3 onto stride-129 diagonals of zeroed dgz
                    nc.scalar.dma_start(
                        dgz[3 * t0:3 * t1].rearrange(
                            "j (p c) -> p j c", p=128, c=129)[:, :, 0],
                        w3bt[:].rearrange("p t k -> p (t k)"))

                def batch_post_b(b):
                    """Diag load (a few tiles after the scatter)."""
                    t0, t1 = NB * b, NB * (b + 1)
                    dg = wrp.tile([128, NB * 3, 128], bf16, tag="dg")
                    dgs[b] = dg
                    nc.scalar.dma_start(
                        dg[:], dgz[3 * t0:3 * t1].rearrange(
                            "j (a c) -> a j c", a=129, c=128)[0:128, :, :])

                def interp_tile(tt):
                    """Weight+transpose one tile's gathered features."""
                    b, j = tt // NB, tt % NB
                    dg = dgs[b]
                    psI = ips.tile([128, 2, 128], f32, tag="psI")
                    for h in range(2):
                        for k in range(3):
                            nc.tensor.matmul(
                                psI[:, h, :],
                                lhsT=g_all[:, 3 * tt + k,
                                           128 * h:128 * (h + 1)],
                                rhs=dg[:, 3 * j + k, :],
                                start=(k == 0), stop=(k == 2))
                    nc.scalar.copy(
                        interp[:, :, 128 * tt:128 * (tt + 1)], psI[:])

                def mlp0_m(b, m):
                    cols = slice(BPTS * b, BPTS * (b + 1))
                    pm = mps.tile([128, BPTS], f32, tag="pm")
                    for ki in range(3):
                        rhs = p1b[:, cols] if ki == 0 else interp[:, ki - 1, cols]
                        nc.tensor.matmul(
                            pm[:], lhsT=w0b[:, ki, 128 * m:128 * (m + 1)],
                            rhs=rhs, start=(ki == 0), stop=(ki == 2))
                    nc.scalar.activation(out=h0[:, m, cols], in_=pm[:],
                                         func=ACT.Relu, bias=b0s[:, m:m + 1])

                def mlp1(b):
                    cols = slice(BPTS * b, BPTS * (b + 1))
                    pm = mps.tile([128, BPTS], f32, tag="pm")
                    for ki in range(2):
                        nc.tensor.matmul(
                            pm[:], lhsT=w1b[:, ki, :], rhs=h0[:, ki, cols],
                            start=(ki == 0), stop=(ki == 1))
                    nc.scalar.activation(out=outsb[:, cols], in_=pm[:],
                                         func=ACT.Relu, bias=b1s[:, 0:1])
                    nc.sync.dma_start(out[:, cols], outsb[:, cols])

                # Emission schedule: batch-b tail work lags its scans by 2-5
                # tiles so its dependencies are ready when the in-order
                # engine queues reach it.
                for t in range(NT):
                    b = t // NB
                    if t % NB == 0 and b >= 2:
                        mlp0_m(b - 2, 0)
                        mlp0_m(b - 2, 1)
                        mlp1(b - 2)
                    if t % NB == 2 and b >= 1:
                        batch_post_b(b - 1)
                    if t % NB == 3 and b >= 1:
                        for _tt in range(NB * (b - 1), NB * b):
                            interp_tile(_tt)
                    knn_tile(t)
                    if t % NB == NB - 1:
                        batch_post_a(b)
                    if t == NT - 1:
                        # batch-2 MLP runs during the final scan/gather window
                        mlp0_m(2, 0)
                        mlp0_m(2, 1)
                        mlp1(2)
                # tail: drain the pipeline
                batch_post_b(NBATCH - 1)
                for _tt in range(NB * (NBATCH - 1), NT):
                    interp_tile(_tt)
                mlp0_m(3, 0)
                mlp0_m(3, 1)
                mlp1(3)

    nc.finalize()
    return nc


def _split3(v):
    h = v.astype(ml_dtypes.bfloat16)
    r = v - h.astype(np.float32)
    m = r.astype(ml_dtypes.bfloat16)
    l = (r - m.astype(np.float32)).astype(ml_dtypes.bfloat16)
    return h, m, l


def make_in_maps(inputs):
    xyz1 = np.asarray(inputs["xyz1"], np.float32)
    xyz2 = np.asarray(inputs["xyz2"], np.float32)
    points1 = np.asarray(inputs["points1"], np.float32)
    points2 = np.asarray(inputs["points2"], np.float32)
    W0 = np.asarray(inputs["W0"], np.float32)
    W1 = np.asarray(inputs["W1"], np.float32)
    b0 = np.asarray(inputs["b0"], np.float32)
    g0 = np.asarray(inputs["g0"], np.float32)
    be0 = np.asarray(inputs["be0"], np.float32)
    m0 = np.asarray(inputs["m0"], np.float32)
    v0 = np.asarray(inputs["v0"], np.float32)
    b1 = np.asarray(inputs["b1"], np.float32)
    g1 = np.asarray(inputs["g1"], np.float32)
    be1 = np.asarray(inputs["be1"], np.float32)
    m1 = np.asarray(inputs["m1"], np.float32)
    v1 = np.asarray(inputs["v1"], np.float32)

    # shared (replicated) operands ------------------------------------
    t2 = 2.0 * xyz2
    s2 = np.sum(xyz2 * xyz2, axis=0)
    t2h, t2m, t2l = _split3(t2)
    s2h, s2m, s2l = _split3(s2)
    aug2 = np.zeros((24, S), dtype=ml_dtypes.bfloat16)
    aug2[0:3], aug2[3:6], aug2[6:9] = t2h, t2m, t2h
    aug2[9:12], aug2[12:15], aug2[15:18] = t2m, t2l, t2h
    aug2[18], aug2[19], aug2[20] = s2h, s2m, s2l

    p2bf = np.ascontiguousarray(points2.T).astype(ml_dtypes.bfloat16)

    # fold BN (inference) into conv weights/biases
    s0 = g0 / np.sqrt(v0 + BN_EPS)
    W0f = W0 * s0[None, :]
    bias0 = (b0 - m0) * s0 + be0
    s1f = g1 / np.sqrt(v1 + BN_EPS)
    W1f = W1 * s1f[None, :]
    bias1 = (b1 - m1) * s1f + be1

    w0h = np.ascontiguousarray(
        W0f.reshape(3, 128, M0).transpose(1, 0, 2)).astype(ml_dtypes.bfloat16)
    w1h = np.ascontiguousarray(
        W1f.reshape(2, 128, M1).transpose(1, 0, 2)).astype(ml_dtypes.bfloat16)
    b0h = np.ascontiguousarray(bias0.reshape(2, 128).T)
    b1h = np.ascontiguousarray(bias1.reshape(1, 128).T)
    dgzh = np.zeros((NT * 3, 128 * 129), dtype=ml_dtypes.bfloat16)

    in_maps = []
    for c in range(NCORES):
        sl = slice(c * NLOC, (c + 1) * NLOC)
        x1 = xyz1[:, sl]
        x1h, x1m, x1l = _split3(x1)
        aug1 = np.zeros((24, NLOC), dtype=ml_dtypes.bfloat16)
        aug1[0:3], aug1[3:6], aug1[6:9] = x1h, x1h, x1m
        aug1[9:12], aug1[12:15], aug1[15:18] = x1m, x1h, x1l
        aug1[18:21] = -1.0
        sq1 = np.sum(x1 * x1, axis=0).reshape(NT, 128).T + 1e-8
        sq1e = np.ascontiguousarray(
            np.repeat(sq1[:, :, None], 3, axis=2).astype(np.float32))
        in_maps.append(dict(
            aug1h=aug1, aug2h=aug2, sq1h=sq1e,
            p1h=np.ascontiguousarray(points1[:, sl]).astype(ml_dtypes.bfloat16),
            p2bf=p2bf, w0h=w0h, w1h=w1h, b0h=b0h, b1h=b1h, dgz=dgzh,
        ))
    return in_maps


def run(inputs, trace=False, **kwargs):
    if "nc" not in _CACHE:
        _CACHE["nc"] = build()
    nc = _CACHE["nc"]
    in_maps = make_in_maps(inputs)
    res = bass_utils.run_bass_kernel_spmd(
        nc, in_maps, core_ids=list(range(NCORES)), trace=trace, **kwargs)
    outs = [res.results[c]["out"] for c in range(NCORES)]
    full = np.concatenate(outs, axis=1)
    return full, res


def kernel(**inputs):
    full, _ = run(inputs, trace=False)
    return full
